# revision 28
# baseline (speedup 1.0000x reference)
"""Causal self-attention (B=4, T=1024, C=768, 12 heads) on 8 trn2 cores.

Sharding: core c = (batch b=c//2, head-group hg=c%2 of 6 heads).
Each core: QKV projection for its head-group (TP column split of Wqkv),
causal attention for 6 heads, partial output projection (TP row split of
Wproj). Host sums the two partials per batch (the all-reduce) and
transposes back.

Device-side layouts (contraction dim always on partitions, so no
on-device transposes are needed):
  x^T  [C=768, T=1024]    (prepared host-side)
  q^T/k^T = W^T x^T as [cols, T]  (lhsT=Wqk slice, rhs=x^T)
  v = x W_v as [T, cols]          (lhsT=x^T chunk, rhs=Wv)
  scoresT [T_k, T_q] = k_h q_h^T  (lhsT=k_h^T, rhs=q_h^T)
  softmax without max-subtraction (scores ~ N(0, 0.1); exp is safe),
  denominator via a ones-column appended to v (row 64 of att@[v|1]),
  out_h^T [64, T_q] = [v|1]^T attT (lhsT=v_ext chunk, rhs=attT chunk)
  proj^T [768, T] = Wp_hg^T out^T (lhsT=Wp slice, rhs=out^T)

Heads run in pairs at SBUF partition offsets 0/64 so the two K=64 QK
matmuls occupy distinct PE row-groups and run concurrently; their score
tiles share one 2-bank PSUM tile so exp is a single ACT op per block.
Staircase (diagonal) blocks are trimmed to the causally-live columns;
the dead upper triangle of the leading 128 cols is zeroed with a DVE
multiply AFTER exp (no PE mask matmuls). The attention phase is
exp(ACT)-bound, so QKV-projection pairs 1/2 and the first projection
half are interleaved into it (qb-major order); PSUM score tiles ring
3-deep so the PE can run ahead of the exp chain. For the LAST pair the
softmax denominator is accumulated early by PE ones-matmuls over att
tiles (row 96 of the widened accumulator) and 1/den is broadcast by a
K=1 PE matmul, shortening the end-of-kernel norm->proj chain. NOTE:
custom DVE ops (reciprocal_approx_fast) require base_partition 0
inputs on HW.
"""

import numpy as np
import ml_dtypes

B, T, C = 4, 1024, 768
NH, HD = 12, 64
HPC = NH // 2          # heads per core = 6
QKCOLS = 2 * HPC * HD  # 768 (q then k cols for this head group)
VC = HPC * HD          # 384
NCORES = 8
TB = 512               # matmul moving free-dim block
BF16 = ml_dtypes.bfloat16

_prog = None


def _build_program():
    import concourse.bass as bass
    import concourse.tile as tile
    from concourse import bacc, mybir

    f32 = mybir.dt.float32
    bf16 = mybir.dt.bfloat16

    nc = bacc.Bacc(
        "TRN2", target_bir_lowering=False, debug=False, enable_asserts=False
    )

    xT = nc.dram_tensor("xT", [C, T], bf16, kind="ExternalInput")
    wqk = nc.dram_tensor("wqk", [C, QKCOLS], bf16, kind="ExternalInput")
    wv = nc.dram_tensor("wv", [C, VC], bf16, kind="ExternalInput")
    wp = nc.dram_tensor("wp", [VC, C], bf16, kind="ExternalInput")
    fconsts = nc.dram_tensor("fconsts", [128, 6 + VC + 6], f32, kind="ExternalInput")
    hconsts = nc.dram_tensor("hconsts", [128, 2, 128], bf16, kind="ExternalInput")
    out = nc.dram_tensor("out", [C, T], bf16, kind="ExternalOutput")

    Exp = mybir.ActivationFunctionType.Exp

    with tile.TileContext(nc) as tc:
        with (
            tc.tile_pool(name="consts", bufs=1) as consts,
            tc.tile_pool(name="psum", bufs=1, space="PSUM") as psum,
            tc.tile_pool(name="work", bufs=1) as work,
        ):
            # ---- SBUF residents ----
            xT_sb = consts.tile([128, 6, T], bf16)
            wv_sb = consts.tile([128, 6, VC], bf16)
            wqk_sb = consts.tile([128, 6, QKCOLS], bf16)
            wp_sb = consts.tile([128, 3, C], bf16)
            fc_sb = consts.tile([128, 6 + VC + 6], f32)
            bqk_sb = fc_sb[:, 0:6]
            bv_sb = fc_sb[:, 6 : 6 + VC]
            bp_sb = fc_sb[:, 6 + VC : 6 + VC + 6]
            tril_sb = consts.tile([128, 2, 128], bf16)  # 0/1 causal mask, x2
            qk_sb = consts.tile([128, 6, T], bf16)   # q^T (blocks 0-2), k^T (3-5)
            v_sb = consts.tile([128, 8, HPC, HD + 1], bf16)  # [Tk chunk][head][v|1]
            out_sb = consts.tile([128, 3, T], bf16)  # attention out^T [384, T]
            wz = consts.tile([128, TB], bf16)

            xT_r = xT.rearrange("(a p) t -> p a t", p=128)
            wqk_r = wqk.rearrange("(a p) c -> p a c", p=128)
            wv_r = wv.rearrange("(a p) c -> p a c", p=128)
            wp_r = wp.rearrange("(a p) c -> p a c", p=128)

            # memsets first so HAM warm-up can start immediately
            nc.vector.memset(wz[:], 0.0)
            nc.vector.memset(v_sb[:, :, :, HD : HD + 1], 1.0)

            # ---- input DMAs: kc-major chase order round-robin over the
            # three queues, wqk interleaved with xT so qk pair 0 can finish
            # right as the tail chunks land; wp (needed mid-attention) last ----
            nc.sync.dma_start(xT_sb[:, 0, :], xT_r[:, 0, :])
            nc.gpsimd.dma_start(wv_sb[:, 0:3, :], wv_r[:, 0:3, :])
            nc.scalar.dma_start(fc_sb[:], fconsts[:])
            nc.scalar.dma_start(wqk_sb[:, 0, :], wqk_r[:, 0, :])
            nc.sync.dma_start(wqk_sb[:, 1, :], wqk_r[:, 1, :])
            nc.gpsimd.dma_start(xT_sb[:, 1, :], xT_r[:, 1, :])
            nc.scalar.dma_start(xT_sb[:, 2, :], xT_r[:, 2, :])
            nc.sync.dma_start(xT_sb[:, 3, :], xT_r[:, 3, :])
            nc.gpsimd.dma_start(wqk_sb[:, 2, :], wqk_r[:, 2, :])
            nc.scalar.dma_start(wqk_sb[:, 3, :], wqk_r[:, 3, :])
            nc.sync.dma_start(xT_sb[:, 4, :], xT_r[:, 4, :])
            nc.gpsimd.dma_start(wv_sb[:, 3:6, :], wv_r[:, 3:6, :])
            nc.scalar.dma_start(xT_sb[:, 5, :], xT_r[:, 5, :])
            nc.sync.dma_start(wqk_sb[:, 4, :], wqk_r[:, 4, :])
            nc.gpsimd.dma_start(wqk_sb[:, 5, :], wqk_r[:, 5, :])
            nc.sync.dma_start(tril_sb[:], hconsts[:])
            nc.gpsimd.dma_start(wp_sb[:], wp_r[:])

            # HAM warm-up: dummy matmuls while the first input DMAs land so
            # the PE clock-gate opens before real work starts. Write-only
            # tiles: the ring slots hand over to wave A with no stall.
            for w in range(6):
                ps_w = psum.tile([128, 2, TB], f32, tag="ps", bufs=3, name="ps_w")
                nc.tensor.matmul(
                    ps_w[:, w % 2, :], wz[:, 0:128], wz[:], start=True, stop=True
                )

            # ---- phase 1a: v = x @ Wv + bv, in [T, cols] layout. tk pairs
            # ping-pong the two PSUM banks so accumulation matmuls pipeline.
            # tkp 0-2 run their kc 0-2 partials first (those chunks land
            # early), so the PE is never head-blocked on the last chunks.
            def v_mms(tkp, ps_v, kc_lo, kc_hi):
                for kc in range(kc_lo, kc_hi):
                    for t2 in range(2):
                        tk = 2 * tkp + t2
                        nc.tensor.matmul(
                            ps_v[:, t2, 0:VC],
                            xT_sb[:, kc, tk * 128 : (tk + 1) * 128],
                            wv_sb[:, kc, :],
                            start=(kc == 0),
                            stop=(kc == 5),
                        )

            def v_evac(tkp, ps_v):
                for t2 in range(2):
                    nc.vector.tensor_add(
                        v_sb[:, 2 * tkp + t2, :, 0:HD],
                        ps_v[:, t2, 0:VC].rearrange("p (h d) -> p h d", h=HPC),
                        bv_sb.rearrange("p (h d) -> p h d", h=HPC),
                    )

            # ---- q^T / k^T = Wqk^T @ x^T, [cols, T]. ----
            def emit_qkproj_cb(cb, kc_lo, kc_hi, ps_box):
                if kc_lo == 0:
                    ps_box[0] = psum.tile(
                        [128, 2, TB], f32, tag="ps", bufs=3, name="ps_qk"
                    )
                ps_qk = ps_box[0]
                for kc in range(kc_lo, kc_hi):
                    for tb in range(2):
                        nc.tensor.matmul(
                            ps_qk[:, tb, :],
                            wqk_sb[:, kc, cb * 128 : (cb + 1) * 128],
                            xT_sb[:, kc, tb * TB : (tb + 1) * TB],
                            start=(kc == 0),
                            stop=(kc == 5),
                        )
                if kc_hi == 6:
                    # always on Scalar: a DVE-queued evac stalls the next
                    # pair's score LDWEIGHTS behind unrelated vector work
                    dst = qk_sb[:, cb, :].rearrange("p (a f) -> p a f", a=2)
                    nc.scalar.add(dst, ps_qk[:], bqk_sb[:, cb : cb + 1])

            # ---- wave A: v for tk 0/1 + BOTH qk pair-0 col-blocks,
            # interleaved kc-major so the PE chases the DMA chunks as they
            # land and pair-0 scores start the moment the tail chunks
            # arrive. v for tk 2-7 (needed only once AVs reach those
            # k-chunks) drips into attention. ----
            ps_va = psum.tile([128, 2, TB], f32, tag="ps", bufs=3, name="ps_va")
            ps_qk0 = psum.tile([128, 2, TB], f32, tag="ps", bufs=3, name="ps_qk0")
            ps_qk3 = psum.tile([128, 2, TB], f32, tag="ps", bufs=3, name="ps_qk3")
            for kc in range(6):
                v_mms(0, ps_va, kc, kc + 1)
                for cb, pst in ((0, ps_qk0), (3, ps_qk3)):
                    for tb in range(2):
                        nc.tensor.matmul(
                            pst[:, tb, :],
                            wqk_sb[:, kc, cb * 128 : (cb + 1) * 128],
                            xT_sb[:, kc, tb * TB : (tb + 1) * TB],
                            start=(kc == 0),
                            stop=(kc == 5),
                        )
            for cb, pst in ((0, ps_qk0), (3, ps_qk3)):
                dst = qk_sb[:, cb, :].rearrange("p (a f) -> p a f", a=2)
                nc.scalar.add(dst, pst[:], bqk_sb[:, cb : cb + 1])
            v_evac(0, ps_va)

            # ---- output projection: 2 output col-blocks per PSUM tile ----
            def emit_proj_obp_mm(obp, tb, r_lo, r_hi, ps_box):
                if r_lo == 0:
                    ps_box[0] = psum.tile(
                        [128, 2, TB], f32, tag="ps", bufs=3, name="ps_pr"
                    )
                ps_pr = ps_box[0]
                for r in range(r_lo, r_hi):
                    for i2 in range(2):
                        ob = 2 * obp + i2
                        nc.tensor.matmul(
                            ps_pr[:, i2, :],
                            wp_sb[:, r, ob * 128 : (ob + 1) * 128],
                            out_sb[:, r, tb * TB : (tb + 1) * TB],
                            start=(r == 0),
                            stop=(r == 2),
                        )

            oeng = [nc.sync, nc.gpsimd]

            def emit_proj_obp_out(obp, tb, ps_box, evac_split=False):
                # GpSimd can't read PSUM; evac on DVE mid-kernel (Scalar is
                # exp-bound there); split Scalar/Vector in the tail (exps
                # done, two lanes drain the 6 blocks twice as fast)
                ps_pr = ps_box[0]
                for i2 in range(2):
                    ob = 2 * obp + i2
                    res = work.tile([128, TB], bf16, tag="res", bufs=4, name="res")
                    if evac_split and ob % 2 == 0:
                        nc.scalar.add(res[:], ps_pr[:, i2, :], bp_sb[:, ob : ob + 1])
                    else:
                        nc.vector.tensor_scalar_add(
                            res[:], ps_pr[:, i2, :], bp_sb[:, ob : ob + 1]
                        )
                    oeng[ob % 2].dma_start(
                        out[ob * 128 : (ob + 1) * 128, tb * TB : (tb + 1) * TB],
                        res[:],
                    )

            # ---- attention, qb-major, with dripped interleave work ----
            extra = []  # deferred interleavable units (qkproj / proj halves)

            def drip():
                if extra:
                    extra.pop(0)()

            def add_qkproj_pair(j):
                for cb in (j, 3 + j):
                    box = [None]
                    extra.append(
                        lambda cb=cb, box=box: emit_qkproj_cb(cb, 0, 3, box)
                    )
                    extra.append(
                        lambda cb=cb, box=box: emit_qkproj_cb(cb, 3, 6, box)
                    )

            def add_v_pair(tkp):
                # self-contained: alloc, full accumulation, evac in one unit
                # so the ring slot is never held across foreign allocations
                def mk(tkp=tkp):
                    ps_vb = psum.tile(
                        [128, 2, TB], f32, tag="ps", bufs=3, name="ps_vb"
                    )
                    v_mms(tkp, ps_vb, 0, 6)
                    v_evac(tkp, ps_vb)

                extra.append(mk)

            def add_proj_half0():
                for obp in range(3):
                    box = [None]
                    extra.append(
                        lambda obp=obp, box=box: emit_proj_obp_mm(obp, 0, 0, 3, box)
                    )
                    extra.append(
                        lambda obp=obp, box=box: emit_proj_obp_out(obp, 0, box)
                    )

            pend = []  # deferred AV / normalization tasks
            norm_last_parts = {}

            for qb in range(2):
                for j in range(3):
                    # drip-queue placement is correctness-relevant: every qk
                    # pair j's last unit (which carries the evac) must pop by
                    # the FIRST drip of phase j (drip precedes the score that
                    # reads it); v-tkp units must pop before the phase whose
                    # deferred AVs read those tk chunks get emitted.
                    if qb == 0 and j == 0:
                        add_qkproj_pair(1)
                    if qb == 0 and j == 1:
                        add_v_pair(1)
                        add_qkproj_pair(2)
                    if qb == 0 and j == 2:
                        add_v_pair(2)
                        add_v_pair(3)
                    if qb == 1 and j == 0:
                        # flush so norm(2,0) is emitted before proj half 0
                        # (which reads out_sb row 2) enters the drip queue
                        while pend:
                            pend.pop(0)()
                        add_proj_half0()
                    last_pair = qb == 1 and j == 2
                    # phases with little interleave work get a throwaway
                    # matmul per block: the HAM clock monitor halves the PE
                    # clock after sustained low activity, which is costlier
                    dummy_fill = (qb == 0 and j == 2) or (qb == 1 and j == 2)
                    qblk, kblk = j, 3 + j
                    hA, hB = 2 * j, 2 * j + 1
                    nkb = 4 * (qb + 1)     # causal: T_k chunks needed
                    oe2 = psum.tile([65, 2, TB], f32, tag="acc", bufs=1, name="oe2")

                    def qk_exp(kb, qblk=qblk, kblk=kblk, qb=qb,
                               dummy_fill=dummy_fill):
                        stair = kb >= qb * 4
                        o = (kb - qb * 4) * 128 if stair else 0
                        qs = slice(qb * TB + o, (qb + 1) * TB)
                        ks = slice(kb * 128, (kb + 1) * 128)
                        ps2 = psum.tile(
                            [128, 2, TB], f32, tag="ps", bufs=3, name="ps2"
                        )
                        if dummy_fill:
                            # discarded: keeps PE activity up for HAM
                            nc.tensor.matmul(
                                ps2[:, 0, o:],
                                wz[:, 0:128],
                                wz[:, o:],
                                start=True,
                                stop=True,
                                skip_group_check=True,
                            )
                        nc.tensor.matmul(
                            ps2[:, 0, o:],
                            qk_sb[0:64, kblk, ks],
                            qk_sb[0:64, qblk, qs],
                            start=True,
                            stop=True,
                        )
                        nc.tensor.matmul(
                            ps2[:, 1, o:],
                            qk_sb[64:128, kblk, ks],
                            qk_sb[64:128, qblk, qs],
                            start=True,
                            stop=True,
                        )
                        att2 = work.tile([128, 2, TB], bf16, tag="att", bufs=6)
                        # exp(score/8); softmax max-subtraction skipped (tiny scores)
                        nc.scalar.activation(
                            att2[:, :, o:], ps2[:, :, o:], Exp, scale=0.125
                        )
                        if stair:
                            # zero the dead upper triangle of the leading 128
                            # cols (the only masked region of a trimmed block)
                            nc.vector.tensor_mul(
                                att2[:, :, o : o + 128],
                                att2[:, :, o : o + 128],
                                tril_sb[:],
                            )
                        return o, att2

                    def av(kb, o, att2, oe2=oe2, hA=hA, hB=hB, nkb=nkb):
                        for i, h in ((0, hA), (1, hB)):
                            nc.tensor.matmul(
                                oe2[:, i, o:],
                                v_sb[:, kb, h, :],
                                att2[:, i, o:],
                                start=(kb == 0),
                                stop=(kb == nkb - 1),
                            )

                    # AV for a block issues only after the next QK (even
                    # across pair boundaries): the PE always has score-matmuls
                    # queued while ACT computes exp, so it never bubbles.
                    for kb in range(nkb):
                        drip()
                        item = (kb, *qk_exp(kb))
                        pend.append(lambda it=item, fn=av: fn(*it))
                        while len(pend) > 2:
                            pend.pop(0)()

                    def norm(oe2=oe2, qblk=qblk, qb=qb):
                        # per-head den/recip chains emitted first (<=0.75us
                        # pieces, pipelined against the GpSimd broadcasts so
                        # the in-order DVE queue never idles on them), then
                        # the accumulator drains via oecp (frees the PSUM
                        # slot fast for the next phase's AVs), then the muls
                        rdbs = []
                        for i in range(2):
                            den2 = work.tile([1, TB], f32, tag="den", bufs=4)
                            nc.vector.tensor_copy(den2[:], oe2[64:65, i, :])
                            rden2 = work.tile([1, TB], f32, tag="rden", bufs=4)
                            nc.vector.reciprocal_approx_fast(rden2[:], den2[:])
                            rdb2 = work.tile([64, TB], f32, tag="rdb", bufs=4)
                            nc.gpsimd.partition_broadcast(rdb2[:], rden2[:])
                            rdbs.append(rdb2)
                        oecp = work.tile([65, 2, TB], f32, tag="oecp", bufs=2)
                        if qb == 0:
                            nc.scalar.copy(oecp[:], oe2[:])
                        else:
                            nc.vector.tensor_copy(oecp[:], oe2[:])
                        for i in range(2):
                            nc.vector.tensor_mul(
                                out_sb[
                                    i * 64 : (i + 1) * 64,
                                    qblk,
                                    qb * TB : (qb + 1) * TB,
                                ],
                                oecp[0:64, i, :],
                                rdbs[i][:],
                            )

                    def norm_last(oe2=oe2, qblk=qblk, qb=qb):
                        # column-split finale: both halves' den/recip/bcast
                        # chains emitted first so they pipeline across the
                        # DVE and GpSimd queues; muls read the accumulator
                        # directly (nothing reuses it after the last pair)
                        rdbs = []
                        for cs in (slice(0, 256), slice(256, TB)):
                            den2 = work.tile(
                                [1, 2, 256], f32, tag="denh", bufs=2, name="den2"
                            )
                            nc.vector.tensor_copy(den2[:], oe2[64:65, :, cs])
                            rden2 = work.tile(
                                [1, 2, 256], f32, tag="rdenh", bufs=2, name="rden2"
                            )
                            nc.vector.reciprocal_approx_fast(
                                rden2[:].rearrange("p a f -> p (a f)"),
                                den2[:].rearrange("p a f -> p (a f)"),
                            )
                            rdb2h = work.tile(
                                [64, 2, 256], f32, tag="rdbh", bufs=2, name="rdb2h"
                            )
                            nc.gpsimd.partition_broadcast(
                                rdb2h[:].rearrange("p a f -> p (a f)"),
                                rden2[:].rearrange("p a f -> p (a f)"),
                            )
                            rdbs.append((cs, rdb2h))
                        for cs, rdb2h in rdbs:
                            for i in range(2):
                                nc.vector.tensor_mul(
                                    out_sb[
                                        i * 64 : (i + 1) * 64,
                                        qblk,
                                        qb * TB + cs.start : qb * TB + cs.stop,
                                    ],
                                    oe2[0:64, i, cs],
                                    rdb2h[:, i, :],
                                )

                    if last_pair:
                        norm_last_parts["fin"] = norm_last
                    else:
                        pend.append(norm)

            while extra:
                drip()          # any leftover proj_half(0) units
            while pend:
                pend.pop(0)()   # remaining deferred AVs (last pair)

            # ---- tail: proj half 1. The last pair's split norm chain
            # overlaps its final AV blocks; r=0/1 accumulate while it runs;
            # r=2 + evac + store close the kernel. Output stores grouped
            # into two 3-block DMAs (issue cost dominates small stores). ----
            norm_last_parts["fin"]()
            boxes = [[None], [None], [None]]
            for obp in range(3):
                emit_proj_obp_mm(obp, 1, 0, 2, boxes[obp])
            for obp in range(3):
                emit_proj_obp_mm(obp, 1, 2, 3, boxes[obp])
                # per-block evac (split Scalar/Vector) + store chase on the
                # two idle DMA queues: the last transfer is 0.125MB instead
                # of 0.75MB, pulling the kernel end forward
                emit_proj_obp_out(obp, 1, boxes[obp], evac_split=True)

    nc.compile()
    return nc


def _get_prog():
    global _prog
    if _prog is None:
        _prog = _build_program()
    return _prog


def make_in_maps(x, Wqkv, bqkv, Wproj, bproj):
    """Host-side sharding: per-core input dict."""
    x = np.asarray(x, dtype=np.float32)
    Wqkv = np.asarray(Wqkv, dtype=np.float32)
    bqkv = np.asarray(bqkv, dtype=np.float32)
    Wproj = np.asarray(Wproj, dtype=np.float32)
    bproj = np.asarray(bproj, dtype=np.float32)

    f = np.arange(128)[None, :]
    p = np.arange(128)[:, None]
    tril = np.where(f >= p, 1.0, 0.0).astype(np.float32)  # [128,128] 0/1
    hconsts = np.concatenate([tril, tril], axis=1).astype(BF16)

    in_maps = []
    for c in range(NCORES):
        b, hg = c // 2, c % 2
        qcols = slice(hg * VC, (hg + 1) * VC)
        kcols = slice(C + hg * VC, C + (hg + 1) * VC)
        vcols = slice(2 * C + hg * VC, 2 * C + (hg + 1) * VC)
        wqk_c = np.concatenate([Wqkv[:, qcols], Wqkv[:, kcols]], axis=1)
        bqk_c = np.concatenate([bqkv[qcols], bqkv[kcols]])
        bp_c = bproj if hg == 0 else np.zeros_like(bproj)
        in_maps.append(
            {
                "xT": np.ascontiguousarray(x[b].T).astype(BF16),
                "wqk": np.ascontiguousarray(wqk_c).astype(BF16),
                "wv": np.ascontiguousarray(Wqkv[:, vcols]).astype(BF16),
                "wp": np.ascontiguousarray(Wproj[hg * VC : (hg + 1) * VC, :]).astype(
                    BF16
                ),
                "fconsts": np.concatenate(
                    [
                        bqk_c.reshape(6, 128).T,
                        np.broadcast_to(bqkv[vcols], (128, VC)),
                        bp_c.reshape(6, 128).T,
                    ],
                    axis=1,
                ).astype(np.float32),
                "hconsts": hconsts.reshape(128, 2, 128),
            }
        )
    return in_maps


def gather_output(results):
    """results: per-core dict with 'out' [768, 1024] partials."""
    outs = []
    for b in range(B):
        part = results[2 * b]["out"].astype(np.float32) + results[2 * b + 1][
            "out"
        ].astype(np.float32)
        outs.append(part.T)
    return np.stack(outs).astype(np.float32)


def run(inputs, trace=False):
    from concourse.bass_utils import run_bass_kernel_spmd

    nc = _get_prog()
    in_maps = make_in_maps(
        inputs["x"], inputs["Wqkv"], inputs["bqkv"], inputs["Wproj"], inputs["bproj"]
    )
    res = run_bass_kernel_spmd(nc, in_maps, list(range(NCORES)), trace=trace)
    return gather_output(res.results), res


def kernel(**inputs):
    out, _ = run(inputs, trace=False)
    return out



# revision 29
# speedup vs baseline: 1.1302x; 1.1302x over previous
"""Causal self-attention (B=4, T=1024, C=768, 12 heads) on 8 trn2 cores.

Sharding: core c = (batch b=c//2, head-group hg=c%2 of 6 heads).
Each core: QKV projection for its head-group (TP column split of Wqkv),
causal attention for 6 heads, partial output projection (TP row split of
Wproj). Host sums the two partials per batch (the all-reduce) and
transposes back.

Device-side layouts (contraction dim always on partitions, so no
on-device transposes are needed):
  x^T  [C=768, T=1024]    (prepared host-side)
  q^T/k^T = W^T x^T as [cols, T]  (lhsT=Wqk slice, rhs=x^T)
  v = x W_v as [T, cols]          (lhsT=x^T chunk, rhs=Wv)
  scoresT [T_k, T_q] = k_h q_h^T  (lhsT=k_h^T, rhs=q_h^T)
  softmax without max-subtraction (scores ~ N(0, 0.1); exp is safe),
  denominator via a ones-column appended to v (row 64 of att@[v|1]),
  out_h^T [64, T_q] = [v|1]^T attT (lhsT=v_ext chunk, rhs=attT chunk)
  proj^T [768, T] = Wp_hg^T out^T (lhsT=Wp slice, rhs=out^T)

Heads run in pairs at SBUF partition offsets 0/64 so the two K=64 QK
matmuls occupy distinct PE row-groups and run concurrently; their score
tiles share one 2-bank PSUM tile so exp is a single ACT op per block.
Staircase (diagonal) blocks are trimmed to the causally-live columns;
the dead upper triangle of the leading 128 cols is zeroed with a DVE
multiply AFTER exp (no PE mask matmuls). The attention phase is
exp(ACT)-bound, so QKV-projection pairs 1/2 and the first projection
half are interleaved into it (qb-major order); PSUM score tiles ring
3-deep so the PE can run ahead of the exp chain. For the LAST pair the
softmax denominator is accumulated early by PE ones-matmuls over att
tiles (row 96 of the widened accumulator) and 1/den is broadcast by a
K=1 PE matmul, shortening the end-of-kernel norm->proj chain. NOTE:
custom DVE ops (reciprocal_approx_fast) require base_partition 0
inputs on HW.
"""

import numpy as np
import ml_dtypes

B, T, C = 4, 1024, 768
NH, HD = 12, 64
HPC = NH // 2          # heads per core = 6
QKCOLS = 2 * HPC * HD  # 768 (q then k cols for this head group)
VC = HPC * HD          # 384
NCORES = 8
TB = 512               # matmul moving free-dim block
BF16 = ml_dtypes.bfloat16

_prog = None


def _build_program():
    import concourse.bass as bass
    import concourse.tile as tile
    from concourse import bacc, mybir

    f32 = mybir.dt.float32
    bf16 = mybir.dt.bfloat16

    nc = bacc.Bacc(
        "TRN2", target_bir_lowering=False, debug=False, enable_asserts=False
    )

    xT = nc.dram_tensor("xT", [C, T], bf16, kind="ExternalInput")
    wqk = nc.dram_tensor("wqk", [C, QKCOLS], bf16, kind="ExternalInput")
    wv = nc.dram_tensor("wv", [C, VC], bf16, kind="ExternalInput")
    wp = nc.dram_tensor("wp", [VC, C], bf16, kind="ExternalInput")
    fconsts = nc.dram_tensor("fconsts", [128, 6 + VC + 6], f32, kind="ExternalInput")
    hconsts = nc.dram_tensor("hconsts", [128, 2, 128], bf16, kind="ExternalInput")
    out = nc.dram_tensor("out", [C, T], bf16, kind="ExternalOutput")

    Exp = mybir.ActivationFunctionType.Exp

    with tile.TileContext(nc) as tc:
        with (
            tc.tile_pool(name="consts", bufs=1) as consts,
            tc.tile_pool(name="psum", bufs=1, space="PSUM") as psum,
            tc.tile_pool(name="work", bufs=1) as work,
        ):
            # ---- SBUF residents ----
            xT_sb = consts.tile([128, 6, T], bf16)
            wv_sb = consts.tile([128, 6, VC], bf16)
            wqk_sb = consts.tile([128, 6, QKCOLS], bf16)
            wp_sb = consts.tile([128, 3, C], bf16)
            fc_sb = consts.tile([128, 6 + VC + 6], f32)
            bqk_sb = fc_sb[:, 0:6]
            bv_sb = fc_sb[:, 6 : 6 + VC]
            bp_sb = fc_sb[:, 6 + VC : 6 + VC + 6]
            tril_sb = consts.tile([128, 2, 128], bf16)  # 0/1 causal mask, x2
            qk_sb = consts.tile([128, 6, T], bf16)   # q^T (blocks 0-2), k^T (3-5)
            v_sb = consts.tile([128, 8, HPC, HD + 1], bf16)  # [Tk chunk][head][v|1]
            out_sb = consts.tile([128, 3, T], bf16)  # attention out^T [384, T]
            wz = consts.tile([128, TB], bf16)

            xT_r = xT.rearrange("(a p) t -> p a t", p=128)
            wqk_r = wqk.rearrange("(a p) c -> p a c", p=128)
            wv_r = wv.rearrange("(a p) c -> p a c", p=128)
            wp_r = wp.rearrange("(a p) c -> p a c", p=128)

            # ---- input DMAs: kc chunks engine-major so kc 0/1/2 land in
            # parallel on the three queues; small consts interleaved early ----
            eng = [nc.sync, nc.gpsimd, nc.scalar]
            for kc in range(6):
                e = eng[kc % 3]
                e.dma_start(xT_sb[:, kc, :], xT_r[:, kc, :])
                e.dma_start(wv_sb[:, kc, :], wv_r[:, kc, :])
                if kc == 2:
                    nc.sync.dma_start(fc_sb[:], fconsts[:])
                    nc.gpsimd.dma_start(tril_sb[:], hconsts[:])
            for kc in range(6):
                eng[kc % 3].dma_start(wqk_sb[:, kc, :], wqk_r[:, kc, :])
            nc.scalar.dma_start(wp_sb[:], wp_r[:])

            nc.vector.memset(v_sb[:, :, :, HD : HD + 1], 1.0)
            nc.vector.memset(wz[:], 0.0)

            # HAM warm-up: dummy matmuls while the first input DMAs land so
            # the PE clock-gate opens before real work starts
            for w in range(10):
                ps_w = psum.tile([128, 2, TB], f32, tag="ps", bufs=3, name="ps_w")
                nc.tensor.matmul(
                    ps_w[:, w % 2, :], wz[:, 0:128], wz[:], start=True, stop=True
                )

            # ---- phase 1a: v = x @ Wv + bv, in [T, cols] layout. tk pairs
            # ping-pong the two PSUM banks so accumulation matmuls pipeline.
            # tkp 0-2 run their kc 0-2 partials first (those chunks land
            # early), so the PE is never head-blocked on the last chunks.
            def v_mms(tkp, ps_v, kc_lo, kc_hi):
                for kc in range(kc_lo, kc_hi):
                    for t2 in range(2):
                        tk = 2 * tkp + t2
                        nc.tensor.matmul(
                            ps_v[:, t2, 0:VC],
                            xT_sb[:, kc, tk * 128 : (tk + 1) * 128],
                            wv_sb[:, kc, :],
                            start=(kc == 0),
                            stop=(kc == 5),
                        )

            def v_evac(tkp, ps_v):
                for t2 in range(2):
                    nc.vector.tensor_add(
                        v_sb[:, 2 * tkp + t2, :, 0:HD],
                        ps_v[:, t2, 0:VC].rearrange("p (h d) -> p h d", h=HPC),
                        bv_sb.rearrange("p (h d) -> p h d", h=HPC),
                    )

            vtiles = []
            for tkp in range(3):
                ps_v = psum.tile([128, 2, TB], f32, tag="ps", bufs=3, name="ps_v")
                vtiles.append(ps_v)
                v_mms(tkp, ps_v, 0, 3)
            for tkp in range(3):
                v_mms(tkp, vtiles[tkp], 3, 6)
                v_evac(tkp, vtiles[tkp])
            ps_v3 = psum.tile([128, 2, TB], f32, tag="ps", bufs=3, name="ps_v3")
            v_mms(3, ps_v3, 0, 6)
            v_evac(3, ps_v3)

            # ---- q^T / k^T = Wqk^T @ x^T, [cols, T]. Pair 0 up front
            # (evac on the still-idle Scalar); pairs 1/2 dripped into the
            # attention phases with evac on Vector. ----
            def emit_qkproj_cb(cb, kc_lo, kc_hi, ps_box):
                if kc_lo == 0:
                    ps_box[0] = psum.tile(
                        [128, 2, TB], f32, tag="ps", bufs=3, name="ps_qk"
                    )
                ps_qk = ps_box[0]
                for kc in range(kc_lo, kc_hi):
                    for tb in range(2):
                        nc.tensor.matmul(
                            ps_qk[:, tb, :],
                            wqk_sb[:, kc, cb * 128 : (cb + 1) * 128],
                            xT_sb[:, kc, tb * TB : (tb + 1) * TB],
                            start=(kc == 0),
                            stop=(kc == 5),
                        )
                if kc_hi == 6:
                    # always on Scalar: a DVE-queued evac stalls the next
                    # pair's score LDWEIGHTS behind unrelated vector work
                    dst = qk_sb[:, cb, :].rearrange("p (a f) -> p a f", a=2)
                    nc.scalar.add(dst, ps_qk[:], bqk_sb[:, cb : cb + 1])

            for cb in (0, 3):
                box = [None]
                emit_qkproj_cb(cb, 0, 6, box)

            # ---- output projection: 2 output col-blocks per PSUM tile ----
            def emit_proj_obp_mm(obp, tb, r_lo, r_hi, ps_box):
                if r_lo == 0:
                    ps_box[0] = psum.tile(
                        [128, 2, TB], f32, tag="ps", bufs=3, name="ps_pr"
                    )
                ps_pr = ps_box[0]
                for r in range(r_lo, r_hi):
                    for i2 in range(2):
                        ob = 2 * obp + i2
                        nc.tensor.matmul(
                            ps_pr[:, i2, :],
                            wp_sb[:, r, ob * 128 : (ob + 1) * 128],
                            out_sb[:, r, tb * TB : (tb + 1) * TB],
                            start=(r == 0),
                            stop=(r == 2),
                        )

            oeng = [nc.sync, nc.gpsimd]

            def emit_proj_obp_out(obp, tb, ps_box, evac_split=False):
                # GpSimd can't read PSUM; evac on DVE mid-kernel (Scalar is
                # exp-bound there); split Scalar/Vector in the tail (exps
                # done, two lanes drain the 6 blocks twice as fast)
                ps_pr = ps_box[0]
                for i2 in range(2):
                    ob = 2 * obp + i2
                    res = work.tile([128, TB], bf16, tag="res", bufs=4, name="res")
                    if evac_split and ob % 2 == 0:
                        nc.scalar.add(res[:], ps_pr[:, i2, :], bp_sb[:, ob : ob + 1])
                    else:
                        nc.vector.tensor_scalar_add(
                            res[:], ps_pr[:, i2, :], bp_sb[:, ob : ob + 1]
                        )
                    oeng[ob % 2].dma_start(
                        out[ob * 128 : (ob + 1) * 128, tb * TB : (tb + 1) * TB],
                        res[:],
                    )

            # ---- attention, qb-major, with dripped interleave work ----
            extra = []  # deferred interleavable units (qkproj / proj halves)

            def drip():
                if extra:
                    extra.pop(0)()

            def add_qkproj_pair(j):
                for cb in (j, 3 + j):
                    box = [None]
                    extra.append(
                        lambda cb=cb, box=box: emit_qkproj_cb(cb, 0, 3, box)
                    )
                    extra.append(
                        lambda cb=cb, box=box: emit_qkproj_cb(cb, 3, 6, box)
                    )

            def add_proj_half0():
                for obp in range(3):
                    box = [None]
                    extra.append(
                        lambda obp=obp, box=box: emit_proj_obp_mm(obp, 0, 0, 3, box)
                    )
                    extra.append(
                        lambda obp=obp, box=box: emit_proj_obp_out(obp, 0, box)
                    )

            pend = []  # deferred AV / normalization tasks
            norm_last_parts = {}

            for qb in range(2):
                for j in range(3):
                    if qb == 0 and j < 2:
                        add_qkproj_pair(j + 1)
                    if qb == 1 and j == 0:
                        # flush so norm(2,0) is emitted before proj half 0
                        # (which reads out_sb row 2) enters the drip queue
                        while pend:
                            pend.pop(0)()
                        add_proj_half0()
                    last_pair = qb == 1 and j == 2
                    # phases with little interleave work get a throwaway
                    # matmul per block: the HAM clock monitor halves the PE
                    # clock after sustained low activity, which is costlier
                    dummy_fill = (qb == 0 and j == 2) or (qb == 1 and j == 2)
                    qblk, kblk = j, 3 + j
                    hA, hB = 2 * j, 2 * j + 1
                    nkb = 4 * (qb + 1)     # causal: T_k chunks needed
                    oe2 = psum.tile([65, 2, TB], f32, tag="acc", bufs=1, name="oe2")

                    def qk_exp(kb, qblk=qblk, kblk=kblk, qb=qb,
                               dummy_fill=dummy_fill):
                        stair = kb >= qb * 4
                        o = (kb - qb * 4) * 128 if stair else 0
                        qs = slice(qb * TB + o, (qb + 1) * TB)
                        ks = slice(kb * 128, (kb + 1) * 128)
                        ps2 = psum.tile(
                            [128, 2, TB], f32, tag="ps", bufs=3, name="ps2"
                        )
                        if dummy_fill:
                            # discarded: keeps PE activity up for HAM
                            nc.tensor.matmul(
                                ps2[:, 0, o:],
                                wz[:, 0:128],
                                wz[:, o:],
                                start=True,
                                stop=True,
                                skip_group_check=True,
                            )
                        nc.tensor.matmul(
                            ps2[:, 0, o:],
                            qk_sb[0:64, kblk, ks],
                            qk_sb[0:64, qblk, qs],
                            start=True,
                            stop=True,
                        )
                        nc.tensor.matmul(
                            ps2[:, 1, o:],
                            qk_sb[64:128, kblk, ks],
                            qk_sb[64:128, qblk, qs],
                            start=True,
                            stop=True,
                        )
                        att2 = work.tile([128, 2, TB], bf16, tag="att", bufs=6)
                        # exp(score/8); softmax max-subtraction skipped (tiny scores)
                        nc.scalar.activation(
                            att2[:, :, o:], ps2[:, :, o:], Exp, scale=0.125
                        )
                        if stair:
                            # zero the dead upper triangle of the leading 128
                            # cols (the only masked region of a trimmed block)
                            nc.vector.tensor_mul(
                                att2[:, :, o : o + 128],
                                att2[:, :, o : o + 128],
                                tril_sb[:],
                            )
                        return o, att2

                    def av(kb, o, att2, oe2=oe2, hA=hA, hB=hB, nkb=nkb):
                        for i, h in ((0, hA), (1, hB)):
                            nc.tensor.matmul(
                                oe2[:, i, o:],
                                v_sb[:, kb, h, :],
                                att2[:, i, o:],
                                start=(kb == 0),
                                stop=(kb == nkb - 1),
                            )

                    # AV for a block issues only after the next QK (even
                    # across pair boundaries): the PE always has score-matmuls
                    # queued while ACT computes exp, so it never bubbles.
                    for kb in range(nkb):
                        drip()
                        item = (kb, *qk_exp(kb))
                        pend.append(lambda it=item, fn=av: fn(*it))
                        while len(pend) > 2:
                            pend.pop(0)()

                    def norm_pre(oe2=oe2):
                        # den straight off the PSUM ones-row, BEFORE the
                        # drain: the recip+broadcast chain starts at last-AV
                        # (den bounced to partition 0 for the custom recip;
                        # broadcast on the otherwise-idle GpSimd)
                        den2 = work.tile([1, 2 * TB], f32, tag="den", bufs=2)
                        nc.vector.tensor_copy(
                            den2[:].rearrange("p (a f) -> p a f", a=2),
                            oe2[64:65, :, :],
                        )
                        rden2 = work.tile([1, 2 * TB], f32, tag="rden", bufs=2)
                        nc.vector.reciprocal_approx_fast(rden2[:], den2[:])
                        rdb2 = work.tile([64, 2 * TB], f32, tag="rdb", bufs=2)
                        nc.gpsimd.partition_broadcast(rdb2[:], rden2[:])
                        return rdb2

                    def norm_mul(rdb2, oe2=oe2, qblk=qblk, qb=qb):
                        # drain engine balances the two pacers: Scalar has
                        # slack in the qb0 phases, Vector in the qb1 ones
                        oecp = work.tile([65, 2, TB], f32, tag="oecp", bufs=2)
                        if qb == 0:
                            nc.scalar.copy(oecp[:], oe2[:])
                        else:
                            nc.vector.tensor_copy(oecp[:], oe2[:])
                        for i in range(2):
                            nc.vector.tensor_mul(
                                out_sb[
                                    i * 64 : (i + 1) * 64,
                                    qblk,
                                    qb * TB : (qb + 1) * TB,
                                ],
                                oecp[0:64, i, :],
                                rdb2[:, i * TB : (i + 1) * TB],
                            )

                    def norm(pre=norm_pre, mul=norm_mul):
                        mul(pre())

                    def norm_last(oe2=oe2, qblk=qblk, qb=qb):
                        # column-split finale: cols [0,256) of the accumulator
                        # are final 2 AV-blocks early (staircase), so their
                        # den/recip/broadcast chain overlaps the last blocks
                        rdbs = []
                        for cs in (slice(0, 256), slice(256, TB)):
                            den2 = work.tile(
                                [1, 2, 256], f32, tag="denh", bufs=2, name="den2"
                            )
                            nc.vector.tensor_copy(den2[:], oe2[64:65, :, cs])
                            rden2 = work.tile(
                                [1, 2, 256], f32, tag="rdenh", bufs=2, name="rden2"
                            )
                            nc.vector.reciprocal_approx_fast(
                                rden2[:].rearrange("p a f -> p (a f)"),
                                den2[:].rearrange("p a f -> p (a f)"),
                            )
                            rdb2h = work.tile(
                                [64, 2, 256], f32, tag="rdbh", bufs=2, name="rdb2h"
                            )
                            nc.gpsimd.partition_broadcast(
                                rdb2h[:].rearrange("p a f -> p (a f)"),
                                rden2[:].rearrange("p a f -> p (a f)"),
                            )
                            rdbs.append((cs, rdb2h))
                        oecp = work.tile([65, 2, TB], f32, tag="oecp", bufs=2)
                        nc.scalar.copy(oecp[:], oe2[:])
                        for cs, rdb2h in rdbs:
                            for i in range(2):
                                nc.vector.tensor_mul(
                                    out_sb[
                                        i * 64 : (i + 1) * 64,
                                        qblk,
                                        qb * TB + cs.start : qb * TB + cs.stop,
                                    ],
                                    oecp[0:64, i, cs],
                                    rdb2h[:, i, :],
                                )

                    if last_pair:
                        norm_last_parts["fin"] = norm_last
                    else:
                        pend.append(norm)

            while extra:
                drip()          # any leftover proj_half(0) units
            while pend:
                pend.pop(0)()   # remaining deferred AVs (last pair)

            # ---- tail: proj half 1. The last pair's split norm chain
            # overlaps its final AV blocks; r=0/1 accumulate while it runs;
            # r=2 + evac + store close the kernel. Output stores grouped
            # into two 3-block DMAs (issue cost dominates small stores). ----
            norm_last_parts["fin"]()
            boxes = [[None], [None], [None]]
            for obp in range(3):
                emit_proj_obp_mm(obp, 1, 0, 2, boxes[obp])
            res3 = [
                work.tile([128, 3, TB], bf16, tag="res3", bufs=2, name="res3a"),
                work.tile([128, 3, TB], bf16, tag="res3", bufs=2, name="res3b"),
            ]
            for obp in range(3):
                emit_proj_obp_mm(obp, 1, 2, 3, boxes[obp])
                ps_pr = boxes[obp][0]
                for i2 in range(2):
                    ob = 2 * obp + i2
                    dst = res3[ob // 3][:, ob % 3, :]
                    if ob % 2 == 0:
                        nc.scalar.add(
                            dst, ps_pr[:, i2, :], bp_sb[:, ob : ob + 1]
                        )
                    else:
                        nc.vector.tensor_scalar_add(
                            dst, ps_pr[:, i2, :], bp_sb[:, ob : ob + 1]
                        )
            nc.sync.dma_start(
                out[0:384, TB : 2 * TB].rearrange("(a p) t -> p a t", p=128),
                res3[0][:],
            )
            nc.gpsimd.dma_start(
                out[384:768, TB : 2 * TB].rearrange("(a p) t -> p a t", p=128),
                res3[1][:],
            )

    nc.compile()
    return nc


def _get_prog():
    global _prog
    if _prog is None:
        _prog = _build_program()
    return _prog


def make_in_maps(x, Wqkv, bqkv, Wproj, bproj):
    """Host-side sharding: per-core input dict."""
    x = np.asarray(x, dtype=np.float32)
    Wqkv = np.asarray(Wqkv, dtype=np.float32)
    bqkv = np.asarray(bqkv, dtype=np.float32)
    Wproj = np.asarray(Wproj, dtype=np.float32)
    bproj = np.asarray(bproj, dtype=np.float32)

    f = np.arange(128)[None, :]
    p = np.arange(128)[:, None]
    tril = np.where(f >= p, 1.0, 0.0).astype(np.float32)  # [128,128] 0/1
    hconsts = np.concatenate([tril, tril], axis=1).astype(BF16)

    in_maps = []
    for c in range(NCORES):
        b, hg = c // 2, c % 2
        qcols = slice(hg * VC, (hg + 1) * VC)
        kcols = slice(C + hg * VC, C + (hg + 1) * VC)
        vcols = slice(2 * C + hg * VC, 2 * C + (hg + 1) * VC)
        wqk_c = np.concatenate([Wqkv[:, qcols], Wqkv[:, kcols]], axis=1)
        bqk_c = np.concatenate([bqkv[qcols], bqkv[kcols]])
        bp_c = bproj if hg == 0 else np.zeros_like(bproj)
        in_maps.append(
            {
                "xT": np.ascontiguousarray(x[b].T).astype(BF16),
                "wqk": np.ascontiguousarray(wqk_c).astype(BF16),
                "wv": np.ascontiguousarray(Wqkv[:, vcols]).astype(BF16),
                "wp": np.ascontiguousarray(Wproj[hg * VC : (hg + 1) * VC, :]).astype(
                    BF16
                ),
                "fconsts": np.concatenate(
                    [
                        bqk_c.reshape(6, 128).T,
                        np.broadcast_to(bqkv[vcols], (128, VC)),
                        bp_c.reshape(6, 128).T,
                    ],
                    axis=1,
                ).astype(np.float32),
                "hconsts": hconsts.reshape(128, 2, 128),
            }
        )
    return in_maps


def gather_output(results):
    """results: per-core dict with 'out' [768, 1024] partials."""
    outs = []
    for b in range(B):
        part = results[2 * b]["out"].astype(np.float32) + results[2 * b + 1][
            "out"
        ].astype(np.float32)
        outs.append(part.T)
    return np.stack(outs).astype(np.float32)


def run(inputs, trace=False):
    from concourse.bass_utils import run_bass_kernel_spmd

    nc = _get_prog()
    in_maps = make_in_maps(
        inputs["x"], inputs["Wqkv"], inputs["bqkv"], inputs["Wproj"], inputs["bproj"]
    )
    res = run_bass_kernel_spmd(nc, in_maps, list(range(NCORES)), trace=trace)
    return gather_output(res.results), res


def kernel(**inputs):
    out, _ = run(inputs, trace=False)
    return out



# revision 30
# speedup vs baseline: 1.1652x; 1.0309x over previous
"""Causal self-attention (B=4, T=1024, C=768, 12 heads) on 8 trn2 cores.

Sharding: core c = (batch b=c//2, head-group hg=c%2 of 6 heads).
Each core: QKV projection for its head-group (TP column split of Wqkv),
causal attention for 6 heads, partial output projection (TP row split of
Wproj). Host sums the two partials per batch (the all-reduce) and
transposes back.

Device-side layouts (contraction dim always on partitions, so no
on-device transposes are needed):
  x^T  [C=768, T=1024]    (prepared host-side)
  q^T/k^T = W^T x^T as [cols, T]  (lhsT=Wqk slice, rhs=x^T)
  v = x W_v as [T, cols]          (lhsT=x^T chunk, rhs=Wv)
  scoresT [T_k, T_q] = k_h q_h^T  (lhsT=k_h^T, rhs=q_h^T)
  softmax without max-subtraction (scores ~ N(0, 0.1); exp is safe),
  denominator via a ones-column appended to v (row 64 of att@[v|1]),
  out_h^T [64, T_q] = [v|1]^T attT (lhsT=v_ext chunk, rhs=attT chunk)
  proj^T [768, T] = Wp_hg^T out^T (lhsT=Wp slice, rhs=out^T)

Heads run in pairs at SBUF partition offsets 0/64 so the two K=64 QK
matmuls occupy distinct PE row-groups and run concurrently; their score
tiles share one 2-bank PSUM tile so exp is a single ACT op per block.
Staircase (diagonal) blocks are trimmed to the causally-live columns;
the dead upper triangle of the leading 128 cols is zeroed with a DVE
multiply AFTER exp (no PE mask matmuls). The attention phase is
exp(ACT)-bound, so QKV-projection pairs 1/2 and the first projection
half are interleaved into it (qb-major order); PSUM score tiles ring
3-deep so the PE can run ahead of the exp chain. For the LAST pair the
softmax denominator is accumulated early by PE ones-matmuls over att
tiles (row 96 of the widened accumulator) and 1/den is broadcast by a
K=1 PE matmul, shortening the end-of-kernel norm->proj chain. NOTE:
custom DVE ops (reciprocal_approx_fast) require base_partition 0
inputs on HW.
"""

import numpy as np
import ml_dtypes

B, T, C = 4, 1024, 768
NH, HD = 12, 64
HPC = NH // 2          # heads per core = 6
QKCOLS = 2 * HPC * HD  # 768 (q then k cols for this head group)
VC = HPC * HD          # 384
NCORES = 8
TB = 512               # matmul moving free-dim block
BF16 = ml_dtypes.bfloat16

_prog = None


def _build_program():
    import concourse.bass as bass
    import concourse.tile as tile
    from concourse import bacc, mybir

    f32 = mybir.dt.float32
    bf16 = mybir.dt.bfloat16

    nc = bacc.Bacc(
        "TRN2", target_bir_lowering=False, debug=False, enable_asserts=False
    )

    xT = nc.dram_tensor("xT", [C, T], bf16, kind="ExternalInput")
    wqk = nc.dram_tensor("wqk", [C, QKCOLS], bf16, kind="ExternalInput")
    wv = nc.dram_tensor("wv", [C, VC], bf16, kind="ExternalInput")
    wp = nc.dram_tensor("wp", [VC, C], bf16, kind="ExternalInput")
    fconsts = nc.dram_tensor("fconsts", [128, 6 + VC + 6], f32, kind="ExternalInput")
    hconsts = nc.dram_tensor("hconsts", [128, 2, 128], bf16, kind="ExternalInput")
    out = nc.dram_tensor("out", [C, T], bf16, kind="ExternalOutput")

    Exp = mybir.ActivationFunctionType.Exp

    with tile.TileContext(nc) as tc:
        with (
            tc.tile_pool(name="consts", bufs=1) as consts,
            tc.tile_pool(name="psum", bufs=1, space="PSUM") as psum,
            tc.tile_pool(name="work", bufs=1) as work,
        ):
            # ---- SBUF residents ----
            xT_sb = consts.tile([128, 6, T], bf16)
            wv_sb = consts.tile([128, 6, VC], bf16)
            wqk_sb = consts.tile([128, 6, QKCOLS], bf16)
            wp_sb = consts.tile([128, 3, C], bf16)
            fc_sb = consts.tile([128, 6 + VC + 6], f32)
            bqk_sb = fc_sb[:, 0:6]
            bv_sb = fc_sb[:, 6 : 6 + VC]
            bp_sb = fc_sb[:, 6 + VC : 6 + VC + 6]
            tril_sb = consts.tile([128, 2, 128], bf16)  # 0/1 causal mask, x2
            qk_sb = consts.tile([128, 6, T], bf16)   # q^T (blocks 0-2), k^T (3-5)
            v_sb = consts.tile([128, 8, HPC, HD + 1], bf16)  # [Tk chunk][head][v|1]
            out_sb = consts.tile([128, 3, T], bf16)  # attention out^T [384, T]
            wz = consts.tile([128, TB], bf16)

            xT_r = xT.rearrange("(a p) t -> p a t", p=128)
            wqk_r = wqk.rearrange("(a p) c -> p a c", p=128)
            wv_r = wv.rearrange("(a p) c -> p a c", p=128)
            wp_r = wp.rearrange("(a p) c -> p a c", p=128)

            # memsets first so HAM warm-up can start immediately
            nc.vector.memset(wz[:], 0.0)
            nc.vector.memset(v_sb[:, :, :, HD : HD + 1], 1.0)

            # ---- input DMAs: kc-major chase order round-robin over the
            # three queues, wqk interleaved with xT so qk pair 0 can finish
            # right as the tail chunks land; wp (needed mid-attention) last ----
            nc.sync.dma_start(xT_sb[:, 0, :], xT_r[:, 0, :])
            nc.gpsimd.dma_start(wv_sb[:, 0:3, :], wv_r[:, 0:3, :])
            nc.scalar.dma_start(fc_sb[:], fconsts[:])
            nc.scalar.dma_start(wqk_sb[:, 0, :], wqk_r[:, 0, :])
            nc.sync.dma_start(wqk_sb[:, 1, :], wqk_r[:, 1, :])
            nc.gpsimd.dma_start(xT_sb[:, 1, :], xT_r[:, 1, :])
            nc.scalar.dma_start(xT_sb[:, 2, :], xT_r[:, 2, :])
            nc.sync.dma_start(xT_sb[:, 3, :], xT_r[:, 3, :])
            nc.gpsimd.dma_start(wqk_sb[:, 2, :], wqk_r[:, 2, :])
            nc.scalar.dma_start(wqk_sb[:, 3, :], wqk_r[:, 3, :])
            nc.sync.dma_start(xT_sb[:, 4, :], xT_r[:, 4, :])
            nc.gpsimd.dma_start(wv_sb[:, 3:6, :], wv_r[:, 3:6, :])
            nc.scalar.dma_start(xT_sb[:, 5, :], xT_r[:, 5, :])
            nc.sync.dma_start(wqk_sb[:, 4, :], wqk_r[:, 4, :])
            nc.gpsimd.dma_start(wqk_sb[:, 5, :], wqk_r[:, 5, :])
            nc.sync.dma_start(tril_sb[:], hconsts[:])
            nc.gpsimd.dma_start(wp_sb[:], wp_r[:])

            # HAM warm-up: dummy matmuls while the first input DMAs land so
            # the PE clock-gate opens before real work starts. Write-only
            # tiles: the ring slots hand over to wave A with no stall.
            for w in range(6):
                ps_w = psum.tile([128, 2, TB], f32, tag="ps", bufs=3, name="ps_w")
                nc.tensor.matmul(
                    ps_w[:, w % 2, :], wz[:, 0:128], wz[:], start=True, stop=True
                )

            # ---- phase 1a: v = x @ Wv + bv, in [T, cols] layout. tk pairs
            # ping-pong the two PSUM banks so accumulation matmuls pipeline.
            # tkp 0-2 run their kc 0-2 partials first (those chunks land
            # early), so the PE is never head-blocked on the last chunks.
            def v_mms(tkp, ps_v, kc_lo, kc_hi):
                for kc in range(kc_lo, kc_hi):
                    for t2 in range(2):
                        tk = 2 * tkp + t2
                        nc.tensor.matmul(
                            ps_v[:, t2, 0:VC],
                            xT_sb[:, kc, tk * 128 : (tk + 1) * 128],
                            wv_sb[:, kc, :],
                            start=(kc == 0),
                            stop=(kc == 5),
                        )

            def v_evac(tkp, ps_v):
                for t2 in range(2):
                    nc.vector.tensor_add(
                        v_sb[:, 2 * tkp + t2, :, 0:HD],
                        ps_v[:, t2, 0:VC].rearrange("p (h d) -> p h d", h=HPC),
                        bv_sb.rearrange("p (h d) -> p h d", h=HPC),
                    )

            # ---- q^T / k^T = Wqk^T @ x^T, [cols, T]. ----
            def emit_qkproj_cb(cb, kc_lo, kc_hi, ps_box):
                if kc_lo == 0:
                    ps_box[0] = psum.tile(
                        [128, 2, TB], f32, tag="ps", bufs=3, name="ps_qk"
                    )
                ps_qk = ps_box[0]
                for kc in range(kc_lo, kc_hi):
                    for tb in range(2):
                        nc.tensor.matmul(
                            ps_qk[:, tb, :],
                            wqk_sb[:, kc, cb * 128 : (cb + 1) * 128],
                            xT_sb[:, kc, tb * TB : (tb + 1) * TB],
                            start=(kc == 0),
                            stop=(kc == 5),
                        )
                if kc_hi == 6:
                    # always on Scalar: a DVE-queued evac stalls the next
                    # pair's score LDWEIGHTS behind unrelated vector work
                    dst = qk_sb[:, cb, :].rearrange("p (a f) -> p a f", a=2)
                    nc.scalar.add(dst, ps_qk[:], bqk_sb[:, cb : cb + 1])

            # ---- wave A: v for tk 0/1 + BOTH qk pair-0 col-blocks,
            # interleaved kc-major so the PE chases the DMA chunks as they
            # land and pair-0 scores start the moment the tail chunks
            # arrive. v for tk 2-7 (needed only once AVs reach those
            # k-chunks) drips into attention. ----
            ps_va = psum.tile([128, 2, TB], f32, tag="ps", bufs=3, name="ps_va")
            ps_qk0 = psum.tile([128, 2, TB], f32, tag="ps", bufs=3, name="ps_qk0")
            ps_qk3 = psum.tile([128, 2, TB], f32, tag="ps", bufs=3, name="ps_qk3")
            for kc in range(6):
                v_mms(0, ps_va, kc, kc + 1)
                for cb, pst in ((0, ps_qk0), (3, ps_qk3)):
                    for tb in range(2):
                        nc.tensor.matmul(
                            pst[:, tb, :],
                            wqk_sb[:, kc, cb * 128 : (cb + 1) * 128],
                            xT_sb[:, kc, tb * TB : (tb + 1) * TB],
                            start=(kc == 0),
                            stop=(kc == 5),
                        )
            for cb, pst in ((0, ps_qk0), (3, ps_qk3)):
                dst = qk_sb[:, cb, :].rearrange("p (a f) -> p a f", a=2)
                nc.scalar.add(dst, pst[:], bqk_sb[:, cb : cb + 1])
            v_evac(0, ps_va)

            # ---- output projection: 2 output col-blocks per PSUM tile ----
            def emit_proj_obp_mm(obp, tb, r_lo, r_hi, ps_box):
                if r_lo == 0:
                    ps_box[0] = psum.tile(
                        [128, 2, TB], f32, tag="ps", bufs=3, name="ps_pr"
                    )
                ps_pr = ps_box[0]
                for r in range(r_lo, r_hi):
                    for i2 in range(2):
                        ob = 2 * obp + i2
                        nc.tensor.matmul(
                            ps_pr[:, i2, :],
                            wp_sb[:, r, ob * 128 : (ob + 1) * 128],
                            out_sb[:, r, tb * TB : (tb + 1) * TB],
                            start=(r == 0),
                            stop=(r == 2),
                        )

            oeng = [nc.sync, nc.gpsimd]

            def emit_proj_obp_out(obp, tb, ps_box, evac_split=False):
                # GpSimd can't read PSUM; evac on DVE mid-kernel (Scalar is
                # exp-bound there); split Scalar/Vector in the tail (exps
                # done, two lanes drain the 6 blocks twice as fast)
                ps_pr = ps_box[0]
                for i2 in range(2):
                    ob = 2 * obp + i2
                    res = work.tile([128, TB], bf16, tag="res", bufs=4, name="res")
                    if evac_split and ob % 2 == 0:
                        nc.scalar.add(res[:], ps_pr[:, i2, :], bp_sb[:, ob : ob + 1])
                    else:
                        nc.vector.tensor_scalar_add(
                            res[:], ps_pr[:, i2, :], bp_sb[:, ob : ob + 1]
                        )
                    oeng[ob % 2].dma_start(
                        out[ob * 128 : (ob + 1) * 128, tb * TB : (tb + 1) * TB],
                        res[:],
                    )

            # ---- attention, qb-major, with dripped interleave work ----
            extra = []  # deferred interleavable units (qkproj / proj halves)

            def drip():
                if extra:
                    extra.pop(0)()

            def add_qkproj_pair(j):
                for cb in (j, 3 + j):
                    box = [None]
                    extra.append(
                        lambda cb=cb, box=box: emit_qkproj_cb(cb, 0, 3, box)
                    )
                    extra.append(
                        lambda cb=cb, box=box: emit_qkproj_cb(cb, 3, 6, box)
                    )

            def add_v_pair(tkp):
                # self-contained: alloc, full accumulation, evac in one unit
                # so the ring slot is never held across foreign allocations
                def mk(tkp=tkp):
                    ps_vb = psum.tile(
                        [128, 2, TB], f32, tag="ps", bufs=3, name="ps_vb"
                    )
                    v_mms(tkp, ps_vb, 0, 6)
                    v_evac(tkp, ps_vb)

                extra.append(mk)

            def add_proj_half0():
                for obp in range(3):
                    box = [None]
                    extra.append(
                        lambda obp=obp, box=box: emit_proj_obp_mm(obp, 0, 0, 3, box)
                    )
                    extra.append(
                        lambda obp=obp, box=box: emit_proj_obp_out(obp, 0, box)
                    )

            pend = []  # deferred AV / normalization tasks
            norm_last_parts = {}

            for qb in range(2):
                for j in range(3):
                    # drip-queue placement is correctness-relevant: every qk
                    # pair j's last unit (which carries the evac) must pop by
                    # the FIRST drip of phase j (drip precedes the score that
                    # reads it); v-tkp units must pop before the phase whose
                    # deferred AVs read those tk chunks get emitted.
                    if qb == 0 and j == 0:
                        add_qkproj_pair(1)
                    if qb == 0 and j == 1:
                        add_v_pair(1)
                        add_qkproj_pair(2)
                    if qb == 0 and j == 2:
                        add_v_pair(2)
                        add_v_pair(3)
                    if qb == 1 and j == 0:
                        # flush so norm(2,0) is emitted before proj half 0
                        # (which reads out_sb row 2) enters the drip queue
                        while pend:
                            pend.pop(0)()
                        add_proj_half0()
                    last_pair = qb == 1 and j == 2
                    # phases with little interleave work get a throwaway
                    # matmul per block: the HAM clock monitor halves the PE
                    # clock after sustained low activity, which is costlier
                    dummy_fill = (qb == 0 and j == 2) or (qb == 1 and j == 2)
                    qblk, kblk = j, 3 + j
                    hA, hB = 2 * j, 2 * j + 1
                    nkb = 4 * (qb + 1)     # causal: T_k chunks needed
                    oe2 = psum.tile([65, 2, TB], f32, tag="acc", bufs=1, name="oe2")

                    def qk_exp(kb, qblk=qblk, kblk=kblk, qb=qb,
                               dummy_fill=dummy_fill):
                        stair = kb >= qb * 4
                        o = (kb - qb * 4) * 128 if stair else 0
                        qs = slice(qb * TB + o, (qb + 1) * TB)
                        ks = slice(kb * 128, (kb + 1) * 128)
                        ps2 = psum.tile(
                            [128, 2, TB], f32, tag="ps", bufs=3, name="ps2"
                        )
                        if dummy_fill:
                            # discarded: keeps PE activity up for HAM
                            nc.tensor.matmul(
                                ps2[:, 0, o:],
                                wz[:, 0:128],
                                wz[:, o:],
                                start=True,
                                stop=True,
                                skip_group_check=True,
                            )
                        nc.tensor.matmul(
                            ps2[:, 0, o:],
                            qk_sb[0:64, kblk, ks],
                            qk_sb[0:64, qblk, qs],
                            start=True,
                            stop=True,
                        )
                        nc.tensor.matmul(
                            ps2[:, 1, o:],
                            qk_sb[64:128, kblk, ks],
                            qk_sb[64:128, qblk, qs],
                            start=True,
                            stop=True,
                        )
                        att2 = work.tile([128, 2, TB], bf16, tag="att", bufs=6)
                        # exp(score/8); softmax max-subtraction skipped (tiny scores)
                        nc.scalar.activation(
                            att2[:, :, o:], ps2[:, :, o:], Exp, scale=0.125
                        )
                        if stair:
                            # zero the dead upper triangle of the leading 128
                            # cols (the only masked region of a trimmed block)
                            nc.vector.tensor_mul(
                                att2[:, :, o : o + 128],
                                att2[:, :, o : o + 128],
                                tril_sb[:],
                            )
                        return o, att2

                    def av(kb, o, att2, oe2=oe2, hA=hA, hB=hB, nkb=nkb):
                        for i, h in ((0, hA), (1, hB)):
                            nc.tensor.matmul(
                                oe2[:, i, o:],
                                v_sb[:, kb, h, :],
                                att2[:, i, o:],
                                start=(kb == 0),
                                stop=(kb == nkb - 1),
                            )

                    # AV for a block issues only after the next QK (even
                    # across pair boundaries): the PE always has score-matmuls
                    # queued while ACT computes exp, so it never bubbles.
                    for kb in range(nkb):
                        drip()
                        item = (kb, *qk_exp(kb))
                        pend.append(lambda it=item, fn=av: fn(*it))
                        while len(pend) > 2:
                            pend.pop(0)()

                    def norm(oe2=oe2, qblk=qblk, qb=qb):
                        # per-head den/recip chains emitted first (<=0.75us
                        # pieces, pipelined against the GpSimd broadcasts so
                        # the in-order DVE queue never idles on them), then
                        # the accumulator drains via oecp (frees the PSUM
                        # slot fast for the next phase's AVs), then the muls
                        rdbs = []
                        for i in range(2):
                            den2 = work.tile([1, TB], f32, tag="den", bufs=4)
                            nc.vector.tensor_copy(den2[:], oe2[64:65, i, :])
                            rden2 = work.tile([1, TB], f32, tag="rden", bufs=4)
                            nc.vector.reciprocal_approx_fast(rden2[:], den2[:])
                            rdb2 = work.tile([64, TB], f32, tag="rdb", bufs=4)
                            nc.gpsimd.partition_broadcast(rdb2[:], rden2[:])
                            rdbs.append(rdb2)
                        oecp = work.tile([65, 2, TB], f32, tag="oecp", bufs=2)
                        if qb == 0:
                            nc.scalar.copy(oecp[:], oe2[:])
                        else:
                            nc.vector.tensor_copy(oecp[:], oe2[:])
                        for i in range(2):
                            nc.vector.tensor_mul(
                                out_sb[
                                    i * 64 : (i + 1) * 64,
                                    qblk,
                                    qb * TB : (qb + 1) * TB,
                                ],
                                oecp[0:64, i, :],
                                rdbs[i][:],
                            )

                    def norm_last(oe2=oe2, qblk=qblk, qb=qb):
                        # column-split finale: both halves' den/recip/bcast
                        # chains emitted first so they pipeline across the
                        # DVE and GpSimd queues; muls read the accumulator
                        # directly (nothing reuses it after the last pair)
                        rdbs = []
                        for cs in (slice(0, 256), slice(256, TB)):
                            den2 = work.tile(
                                [1, 2, 256], f32, tag="denh", bufs=2, name="den2"
                            )
                            nc.vector.tensor_copy(den2[:], oe2[64:65, :, cs])
                            rden2 = work.tile(
                                [1, 2, 256], f32, tag="rdenh", bufs=2, name="rden2"
                            )
                            nc.vector.reciprocal_approx_fast(
                                rden2[:].rearrange("p a f -> p (a f)"),
                                den2[:].rearrange("p a f -> p (a f)"),
                            )
                            rdb2h = work.tile(
                                [64, 2, 256], f32, tag="rdbh", bufs=2, name="rdb2h"
                            )
                            nc.gpsimd.partition_broadcast(
                                rdb2h[:].rearrange("p a f -> p (a f)"),
                                rden2[:].rearrange("p a f -> p (a f)"),
                            )
                            rdbs.append((cs, rdb2h))
                        for cs, rdb2h in rdbs:
                            for i in range(2):
                                nc.vector.tensor_mul(
                                    out_sb[
                                        i * 64 : (i + 1) * 64,
                                        qblk,
                                        qb * TB + cs.start : qb * TB + cs.stop,
                                    ],
                                    oe2[0:64, i, cs],
                                    rdb2h[:, i, :],
                                )

                    if last_pair:
                        norm_last_parts["fin"] = norm_last
                    else:
                        pend.append(norm)

            while extra:
                drip()          # any leftover proj_half(0) units
            while pend:
                pend.pop(0)()   # remaining deferred AVs (last pair)

            # ---- tail: proj half 1. The last pair's split norm chain
            # overlaps its final AV blocks; r=0/1 accumulate while it runs;
            # r=2 + evac + store close the kernel. Output stores grouped
            # into two 3-block DMAs (issue cost dominates small stores). ----
            norm_last_parts["fin"]()
            boxes = [[None], [None], [None]]
            for obp in range(3):
                emit_proj_obp_mm(obp, 1, 0, 2, boxes[obp])
            for obp in range(3):
                emit_proj_obp_mm(obp, 1, 2, 3, boxes[obp])
                # per-block evac (split Scalar/Vector) + store chase on the
                # two idle DMA queues: the last transfer is 0.125MB instead
                # of 0.75MB, pulling the kernel end forward
                emit_proj_obp_out(obp, 1, boxes[obp], evac_split=True)

    nc.compile()
    return nc


def _get_prog():
    global _prog
    if _prog is None:
        _prog = _build_program()
    return _prog


def make_in_maps(x, Wqkv, bqkv, Wproj, bproj):
    """Host-side sharding: per-core input dict."""
    x = np.asarray(x, dtype=np.float32)
    Wqkv = np.asarray(Wqkv, dtype=np.float32)
    bqkv = np.asarray(bqkv, dtype=np.float32)
    Wproj = np.asarray(Wproj, dtype=np.float32)
    bproj = np.asarray(bproj, dtype=np.float32)

    f = np.arange(128)[None, :]
    p = np.arange(128)[:, None]
    tril = np.where(f >= p, 1.0, 0.0).astype(np.float32)  # [128,128] 0/1
    hconsts = np.concatenate([tril, tril], axis=1).astype(BF16)

    in_maps = []
    for c in range(NCORES):
        b, hg = c // 2, c % 2
        qcols = slice(hg * VC, (hg + 1) * VC)
        kcols = slice(C + hg * VC, C + (hg + 1) * VC)
        vcols = slice(2 * C + hg * VC, 2 * C + (hg + 1) * VC)
        wqk_c = np.concatenate([Wqkv[:, qcols], Wqkv[:, kcols]], axis=1)
        bqk_c = np.concatenate([bqkv[qcols], bqkv[kcols]])
        bp_c = bproj if hg == 0 else np.zeros_like(bproj)
        in_maps.append(
            {
                "xT": np.ascontiguousarray(x[b].T).astype(BF16),
                "wqk": np.ascontiguousarray(wqk_c).astype(BF16),
                "wv": np.ascontiguousarray(Wqkv[:, vcols]).astype(BF16),
                "wp": np.ascontiguousarray(Wproj[hg * VC : (hg + 1) * VC, :]).astype(
                    BF16
                ),
                "fconsts": np.concatenate(
                    [
                        bqk_c.reshape(6, 128).T,
                        np.broadcast_to(bqkv[vcols], (128, VC)),
                        bp_c.reshape(6, 128).T,
                    ],
                    axis=1,
                ).astype(np.float32),
                "hconsts": hconsts.reshape(128, 2, 128),
            }
        )
    return in_maps


def gather_output(results):
    """results: per-core dict with 'out' [768, 1024] partials."""
    outs = []
    for b in range(B):
        part = results[2 * b]["out"].astype(np.float32) + results[2 * b + 1][
            "out"
        ].astype(np.float32)
        outs.append(part.T)
    return np.stack(outs).astype(np.float32)


def run(inputs, trace=False):
    from concourse.bass_utils import run_bass_kernel_spmd

    nc = _get_prog()
    in_maps = make_in_maps(
        inputs["x"], inputs["Wqkv"], inputs["bqkv"], inputs["Wproj"], inputs["bproj"]
    )
    res = run_bass_kernel_spmd(nc, in_maps, list(range(NCORES)), trace=trace)
    return gather_output(res.results), res


def kernel(**inputs):
    out, _ = run(inputs, trace=False)
    return out

